# revision 2
# baseline (speedup 1.0000x reference)
"""Bass/Tile TRN2 kernel for nn_Link_83047487635827 (gnn_message_passing).

Math (verified against the reference):
    binary = (tag_to_token > 0)                       # (T, N)
    temp   = relu(C^T @ binary),  C = I - strict_lower_ones(T)
    r      = rowsum(temp); P = temp @ inputs          # (T,), (T, D)
    child  == gat_mask  (reference deduce_child is an identity for 0/1 masks)
    out    = (I - S_up)^{-1} @ L_low @ diag(1/r) @ P
    (I - S_up)^{-1} = prod_{k=0..6} (I + S_up^(2^k))   # S_up nilpotent

Sharding (tensor parallel over D, per the hint): every core loads the FULL
binarized tag_to_token (bf16, exact for a 0/1 mask) and redundantly computes
temp, but only its own 128-column slice of x (bf16) and P.  A ones-column
appended to x yields r in the same PSUM accumulation.  No collective at all;
each core writes its (T, 128) output slice and the host concatenates.
temp entries are 0/1 and C is 0/+-1, so bf16 mask matmuls are exact; the
only approximation is bf16 rounding of x (~0.4% << the 2e-2 tolerance).
"""

import numpy as np

B, T, N, D = 1, 128, 32768, 1024
NCORES = 8
DS = D // NCORES          # output columns per core = 128
CHUNK = 512               # tokens per pipeline chunk
NCHUNK = N // CHUNK       # 64
SUBS = CHUNK // 128       # 4 token-subtiles of 128 per chunk
XW = DS + 1               # x slice + ones column = 129
NSUB = N // 128           # 256 subtiles total

_PROGRAM = {}             # (with_cc, loop_stream) -> nc


def _host_consts():
    f32 = np.float32
    ident = np.eye(T, dtype=f32)
    # C[j, i] = 1 if j == i, -1 if j > i  (temp^T tile = binary_tile^T @ C)
    cmat = np.eye(T, dtype=f32) - np.tril(np.ones((T, T), dtype=f32), -1)
    msl = np.tril(np.ones((T, T), dtype=f32), -1)   # strict lower
    msu = np.triu(np.ones((T, T), dtype=f32), 1)    # strict upper
    mle = np.tril(np.ones((T, T), dtype=f32), 0)    # lower inclusive
    import ml_dtypes

    return {
        "ident": ident, "cmat": cmat.astype(ml_dtypes.bfloat16),
        "msl": msl, "msu": msu, "mle": mle,
    }


def _build_program(with_cc=True, loop_stream=1):
    import contextlib

    import concourse.bacc as bacc
    import concourse.bass as bass
    import concourse.mybir as mybir
    import concourse.tile as tile
    from concourse.bass import ts

    f32 = mybir.dt.float32
    bf16 = mybir.dt.bfloat16
    i32 = mybir.dt.int32
    Alu = mybir.AluOpType

    nc = bacc.Bacc(
        "TRN2", target_bir_lowering=False, debug=False, num_devices=NCORES
    )

    # x slice, host-permuted: row j*128+p, col s*XW+q = x_aug[(j*SUBS+s)*128+p, q]
    xs_d = nc.dram_tensor("xs", (NCHUNK * 128, SUBS * XW), bf16, kind="ExternalInput")
    t2t_d = nc.dram_tensor("t2t", (T, N), bf16, kind="ExternalInput")
    gm_d = nc.dram_tensor("gm", (T, T), i32, kind="ExternalInput")
    ident_d = nc.dram_tensor("ident", (T, T), f32, kind="ExternalInput")
    cmat_d = nc.dram_tensor("cmat", (T, T), bf16, kind="ExternalInput")
    msl_d = nc.dram_tensor("msl", (T, T), f32, kind="ExternalInput")
    msu_d = nc.dram_tensor("msu", (T, T), f32, kind="ExternalInput")
    mle_d = nc.dram_tensor("mle", (T, T), f32, kind="ExternalInput")
    out_d = nc.dram_tensor("out", (T, DS), f32, kind="ExternalOutput")

    with tile.TileContext(nc) as tc:
        with (
            tc.tile_pool(name="const", bufs=1) as constp,
            tc.tile_pool(name="xin", bufs=4) as xp,
            tc.tile_pool(name="t2tin", bufs=4) as t2tp,
            tc.tile_pool(name="work", bufs=4) as workp,
            tc.tile_pool(name="mchain", bufs=2) as mp,
            tc.tile_pool(name="psacc", bufs=1, space=bass.MemorySpace.PSUM) as psA,
            tc.tile_pool(name="pstt", bufs=2, space=bass.MemorySpace.PSUM) as psB,
            tc.tile_pool(name="psm", bufs=3, space=bass.MemorySpace.PSUM) as psM,
        ):
            # ---- constants ----
            ident = constp.tile([T, T], f32, tag="ident")
            nc.sync.dma_start(ident[:], ident_d[:])
            cmat = constp.tile([T, T], bf16, tag="cmat")
            nc.sync.dma_start(cmat[:], cmat_d[:])
            msl = constp.tile([T, T], f32, tag="msl")
            nc.sync.dma_start(msl[:], msl_d[:])
            msu = constp.tile([T, T], f32, tag="msu")
            nc.sync.dma_start(msu[:], msu_d[:])
            mle = constp.tile([T, T], f32, tag="mle")
            nc.sync.dma_start(mle[:], mle_d[:])
            gm_i = constp.tile([T, T], i32, tag="gmi")
            nc.sync.dma_start(gm_i[:], gm_d[:])
            gm_f = constp.tile([T, T], f32, tag="gmf")
            nc.vector.tensor_copy(gm_f[:], gm_i[:])

            # ---- recurrence matrix chain (tiny; overlaps the stream loop) ----
            gmT_ps = psM.tile([T, T], f32, tag="mm")
            nc.tensor.transpose(gmT_ps[:], gm_f[:], ident[:])
            gmT = constp.tile([T, T], f32, tag="gmT")
            nc.vector.tensor_copy(gmT[:], gmT_ps[:])

            Tp = mp.tile([T, T], f32, tag="Tp")
            nc.vector.tensor_tensor(out=Tp[:], in0=gmT[:], in1=msl[:], op=Alu.mult)
            TpT = mp.tile([T, T], f32, tag="TpT")
            nc.vector.tensor_tensor(out=TpT[:], in0=gm_f[:], in1=msu[:], op=Alu.mult)
            G = mp.tile([T, T], f32, tag="G")
            nc.vector.tensor_tensor(out=G[:], in0=ident[:], in1=Tp[:], op=Alu.add)
            L_low = constp.tile([T, T], f32, tag="Llow")
            nc.vector.tensor_tensor(out=L_low[:], in0=gm_f[:], in1=mle[:], op=Alu.mult)

            for _k in range(1, 7):
                # matmul(out, lhsT, rhs) = lhsT.T @ rhs
                sq_ps = psM.tile([T, T], f32, tag="mm")
                nc.tensor.matmul(sq_ps[:], Tp[:], TpT[:])      # (Tp^2)^T
                sq2_ps = psM.tile([T, T], f32, tag="mm")
                nc.tensor.matmul(sq2_ps[:], TpT[:], Tp[:])     # Tp^2
                Tp_n = mp.tile([T, T], f32, tag="Tp")
                nc.vector.tensor_copy(Tp_n[:], sq2_ps[:])
                TpT_n = mp.tile([T, T], f32, tag="TpT")
                nc.vector.tensor_copy(TpT_n[:], sq_ps[:])
                gu_ps = psM.tile([T, T], f32, tag="mm")
                nc.tensor.matmul(gu_ps[:], TpT_n[:], G[:])     # Tp^2 @ G
                G_n = mp.tile([T, T], f32, tag="G")
                nc.vector.tensor_tensor(out=G_n[:], in0=G[:], in1=gu_ps[:], op=Alu.add)
                Tp, TpT, G = Tp_n, TpT_n, G_n

            mt_ps = psM.tile([T, T], f32, tag="mm")
            nc.tensor.matmul(mt_ps[:], L_low[:], G[:])         # M^T = L_low^T @ G
            MT = constp.tile([T, T], f32, tag="MT")
            nc.vector.tensor_copy(MT[:], mt_ps[:])

            # ---- streaming phase: P_aug[tag, :DS] += temp @ x_slice,
            #      P_aug[tag, DS] += rowsum(temp) via the ones column ----
            loop_cm = (
                tc.For_i(0, loop_stream, 1)
                if loop_stream > 1
                else contextlib.nullcontext()
            )
            with loop_cm:
                P_ps = psA.tile([128, XW], f32, tag="pacc")

                for j in range(NCHUNK):
                    tt_in = t2tp.tile([T, CHUNK], bf16, tag="ttin")
                    nc.sync.dma_start(tt_in[:], t2t_d[:, ts(j, CHUNK)])
                    xt = xp.tile([128, SUBS * XW], bf16, tag="xt")
                    nc.sync.dma_start(xt[:], xs_d[ts(j, 128), :])

                    ttp = psB.tile([128, CHUNK], f32, tag="tt")
                    for s in range(SUBS):
                        nc.tensor.matmul(
                            ttp[:, ts(s, 128)], tt_in[:, ts(s, 128)], cmat[:]
                        )
                    tempT = workp.tile([128, CHUNK], bf16, tag="tempT")
                    nc.vector.tensor_scalar_max(tempT[:], ttp[:], 0.0)  # relu

                    for s in range(SUBS):
                        i = j * SUBS + s
                        nc.tensor.matmul(
                            P_ps[:],
                            tempT[:, ts(s, 128)],
                            xt[:, ts(s, XW)],
                            start=(i == 0),
                            stop=(i == NSUB - 1),
                        )

            # ---- out = M @ (diag(1/r) P)  (lhsT = MT) ----
            P_sb = workp.tile([128, XW], f32, tag="Psb")
            nc.vector.tensor_copy(P_sb[:], P_ps[:])
            inv_r = workp.tile([128, 1], f32, tag="invr")
            nc.vector.reciprocal(inv_r[:], P_sb[:, DS : DS + 1])
            nc.vector.tensor_scalar_mul(P_sb[:, 0:DS], P_sb[:, 0:DS], inv_r[:])

            o_ps = psB.tile([128, CHUNK], f32, tag="tt")
            nc.tensor.matmul(o_ps[:, 0:DS], MT[:], P_sb[:, 0:DS])
            out_sb = workp.tile([128, DS], f32, tag="outsb")
            nc.vector.tensor_copy(out_sb[:], o_ps[:, 0:DS])
            nc.sync.dma_start(out_d[:], out_sb[:])

    nc.compile()
    return nc


def _get_program(with_cc=True, loop_stream=1):
    key = (with_cc, loop_stream)
    if key not in _PROGRAM:
        _PROGRAM[key] = _build_program(with_cc, loop_stream)
    return _PROGRAM[key]


def _make_in_maps(inputs):
    import ml_dtypes

    bf16 = ml_dtypes.bfloat16
    x = np.asarray(inputs["inputs"], dtype=np.float32).reshape(N, D)
    t2t = np.asarray(inputs["tag_to_token"], dtype=np.float32).reshape(T, N)
    gm = np.asarray(inputs["gat_mask"], dtype=np.int32).reshape(T, T)
    t2t_bin = (t2t > 0).astype(bf16)
    consts = _host_consts()
    in_maps = []
    for c in range(NCORES):
        xc = x[:, c * DS : (c + 1) * DS].astype(bf16)
        xa = np.concatenate([xc, np.ones((N, 1), dtype=bf16)], axis=1)
        # (N, XW) -> chunk-major layout: [j*128+p, s*XW+q] = xa[(j*SUBS+s)*128+p, q]
        xa = np.ascontiguousarray(
            xa.reshape(NCHUNK, SUBS, 128, XW)
            .transpose(0, 2, 1, 3)
            .reshape(NCHUNK * 128, SUBS * XW)
        )
        m = {"xs": xa, "t2t": t2t_bin, "gm": gm}
        m.update(consts)
        in_maps.append(m)
    return in_maps


def _run(inputs, trace=False, **kw):
    from concourse.bass_utils import run_bass_kernel_spmd

    nc = _get_program()
    in_maps = _make_in_maps(inputs)
    res = run_bass_kernel_spmd(
        nc, in_maps, list(range(NCORES)), trace=trace, **kw
    )
    out = np.empty((T, D), dtype=np.float32)
    for c in range(NCORES):
        out[:, c * DS : (c + 1) * DS] = np.asarray(res.results[c]["out"])
    return out.reshape(B, T, D), res


def kernel(**inputs) -> np.ndarray:
    out, _ = _run(inputs, trace=False)
    return out
